# revision 106
# baseline (speedup 1.0000x reference)
"""Minibatch discrimination kernel for 8 Trainium2 NeuronCores.

Math (reference):
    M = einsum('bi,iok->bok', x, T)            # [B, O, K]
    norm[i, j, o] = sum_k |M[i,o,k] - M[j,o,k]|
    out[j, o] = sum_i exp(-norm[i,j,o]) - 1.0  # [B, O]

Strategy (v2):
  - SPMD across 8 cores; core c receives xT rotated by -64*c rows
    (pre-transposed on host), so its local rows 0..63 are global rows
    64c..64c+63. Only the first 320 b-columns of MT are ever read.
  - Per core, MT = (x @ T)^T in 4 o-groups of [125 = (25 o, 5 k), 320 b].
  - Symmetry: each unordered pair {a, b} evaluated exactly once. Core c,
    local row j, covers ring distances d = 1..255 (window [j+1, j+256)).
    The d=256 pairs are handled in one small batched pass, masked to
    cores 0-3 via a per-core exp-bias input (EXB = 0 or -40).
  - |MT - col_j| in ONE op per (j, group): a custom fused DVE op
    ABS_DIFF_COL_ANT = maxx(Src0 - C3, C3 - Src0) with the column
    latched from in1, or ScalarE Abs(scale=-1, bias=col); split between
    the two engines by a static schedule. fp16 output feeds one +BO
    matmul per group (k-reduction, tile_position col 32g).
  - exp: one ACT Exp per j with accum_out row-sums into OB. Column
    contributions accumulate into EAC on GpSimd (tensor_tensor add).
  - Host assembles OB row sums + EAC column sums.
"""

import numpy as np

import concourse.bass as bass
import concourse.bacc as bacc
import concourse.mybir as mybir
from concourse.tile import TileContext
from concourse.bass_utils import run_bass_kernel_spmd

B = 512
IN_F = 512
O = 100
K = 5
NCORES = 8
JB = B // NCORES          # 64 output rows per core
NG = 4                    # o-groups
OG = O // NG              # 25 o's per group
PG = OG * K               # 125 partitions per group
W = 255                   # ring window width (d = 1..255)
XC = 320                  # MT b-columns actually used (max index 319)
F32 = mybir.dt.float32
F16 = mybir.dt.float16

# |d| op engine schedule: counts out of the 256 (j, group) ops.
# D = DVE custom abs-diff (1 op), S = ScalarE Abs, G = GpSimd relu-pair
# (2 ops + extra matmul, needs BON).
N_DVE = 174
N_GPS = 0
N_SCL = 256 - N_DVE - N_GPS
PIPE_DEPTH = 4
DG_BUFS = 8
DG2_BUFS = 10
NP_BUFS = 5
EP_BUFS = 5
BF16 = mybir.dt.bfloat16  # MT via bf16x2 split: xh@Th + xl@Th + xh@Tl


def _register_op(name, spec, subdim):
    """Register a custom DVE op (idempotent)."""
    from concourse import dve_ops
    from concourse.dve_spec import lower, _has_src1
    from concourse.dve_uop import DveOpSpec

    for op in dve_ops.OPS:
        if op.name == name:
            return op
    opcode = 1 + len(dve_ops.OPS)
    shas = {}
    for ver in ("v3", "v4"):
        uops = lower(spec, ver=ver)
        ospec = DveOpSpec(name=name, opcode=opcode, uops=uops,
                          rd1_en=_has_src1(spec))
        shas[ver] = ospec.sha(ver)
    op = dve_ops.DveOp(name, spec, subdim=subdim, uops_sha=shas)
    dve_ops.OPS.append(op)
    dve_ops.CUSTOM_DVE_SPECS[name] = spec
    dve_ops._SUB_OPCODE_FOR_NAME[name] = opcode
    return op


def _make_ops():
    from concourse.dve_spec import (
        Spec, Src0, C0, C1, One, maxx, select, SubIdx)

    single = _register_op(
        "ABS_DIFF_COL_ANT",
        Spec(
            body=maxx(Src0 - C0, C0 - Src0),
            reference=lambda in0, in1, s0, s1, imm2: np.abs(
                in0.astype(np.float32) - s0),
        ),
        subdim=False,
    )

    sel = select(SubIdx >= One, C1, C0)

    def _ref2(in0, in1, s0, s1, imm2):
        x = in0.astype(np.float32)
        out = np.empty_like(x)
        out[:, 0, :] = np.abs(x[:, 0, :] - s0)
        out[:, 1, :] = np.abs(x[:, 1, :] - s1)
        return out

    dual = _register_op(
        "ABS_DIFF_2COL_ANT",
        Spec(body=maxx(Src0 - sel, sel - Src0), reference=_ref2),
        subdim=True,
    )
    return single, dual


ABS_DIFF_COL, ABS_DIFF_2COL = _make_ops()


def _make_schedule(n_dve=N_DVE, n_gps=N_GPS, n_scl=N_SCL):
    """Spread engine tags over the 256 (j,g) ops."""
    assert n_dve + n_gps + n_scl == 256
    tags = []
    acc = {"D": 0.0, "G": 0.0, "S": 0.0}
    quota = {"D": n_dve, "G": n_gps, "S": n_scl}
    cnt = {"D": 0, "G": 0, "S": 0}
    for t in range(256):
        for e in ("D", "G", "S"):
            acc[e] += quota[e] / 256.0
        if t < 12:
            e = "D"      # first j's all-DVE: ScalarE busy with table load
        else:
            e = max(("D", "G", "S"), key=lambda x: acc[x] - cnt[x])
        cnt[e] += 1
        tags.append(e)
    # Cluster S-tags within each j: a 2-S row then breaks only ONE group
    # pair into singles, keeping the other as a fused dual op.
    for j in range(256 // 4):
        t4 = tags[4 * j:4 * j + 4]
        ns = t4.count("S")
        if ns:
            tags[4 * j:4 * j + 4] = ["S"] * ns + [x for x in t4 if x != "S"]
    return tags


def _build_nc(sched=None):
    nc = bacc.Bacc()
    if sched is None:
        sched = _make_schedule()
    has_gps = "G" in sched

    xh = nc.declare_dram_parameter("XH", [IN_F, XC], BF16, isOutput=False)
    xl = nc.declare_dram_parameter("XL", [IN_F, XC], BF16, isOutput=False)
    th = nc.declare_dram_parameter("TH", [IN_F, O * K], BF16, isOutput=False)
    tl = nc.declare_dram_parameter("TL", [IN_F, O * K], BF16, isOutput=False)
    bo = nc.declare_dram_parameter("BO", [PG, 32], F16, isOutput=False)
    bon = nc.declare_dram_parameter("BON", [PG, 32], F16, isOutput=False)
    exb = nc.declare_dram_parameter("EXB", [128, 1], F32, isOutput=False)
    ob = nc.declare_dram_parameter("OB", [128, JB], F32, isOutput=True)
    eaca = nc.declare_dram_parameter("EACA", [128, XC], F32, isOutput=True)
    eacb = nc.declare_dram_parameter("EACB", [128, XC], F32, isOutput=True)

    with TileContext(nc) as tc:
        with (
            tc.tile_pool(name="const", bufs=1) as cpool,
            tc.tile_pool(name="dg", bufs=DG_BUFS) as dgpool,
            tc.tile_pool(name="dg2", bufs=DG2_BUFS) as dg2pool,
            tc.tile_pool(name="mps", bufs=2, space="PSUM") as mpspool,
            tc.tile_pool(name="nps", bufs=NP_BUFS, space="PSUM") as npspool,
            tc.tile_pool(name="npx", bufs=1, space="PSUM") as npxpool,
            tc.tile_pool(name="eps", bufs=EP_BUFS) as eppool,
        ):
            bo_sb = cpool.tile([PG, 32], F16, name="bo_sb")
            nc.sync.dma_start(out=bo_sb[:], in_=bo[:])
            if has_gps:
                bon_sb = cpool.tile([PG, 32], F16, name="bon_sb")
                nc.sync.dma_start(out=bon_sb[:], in_=bon[:])
            exb_sb = cpool.tile([128, 1], F32, name="exb_sb")
            nc.sync.dma_start(out=exb_sb[:], in_=exb[:])

            # Single mega-DMA per input: [512, C] -> [128, 4*C] with the
            # 4 row-chunks side by side (one descriptor issue each).
            # Input DMAs in two halves (it 0-1, then it 2-3), X on the
            # scalar queue / T on sync, so the it01 MT matmuls overlap the
            # it23 transfers.
            srcs = [(xh, XC, "xh_all", nc.scalar), (xl, XC, "xl_all", nc.scalar),
                    (th, O * K, "th_all", nc.sync), (tl, O * K, "tl_all", nc.sync)]
            tiles = {n: cpool.tile([128, 4 * c], BF16, name=n)
                     for _, c, n, _q in srcs}
            for h in range(2):
                for dram, cols, name, q in srcs:
                    t = tiles[name]
                    q.dma_start(
                        out=t[:, h * 2 * cols:(h + 1) * 2 * cols].rearrange(
                            "p (i c) -> p i c", i=2),
                        in_=dram[h * 256:(h + 1) * 256, :].rearrange(
                            "(i p) c -> p i c", p=128))
            xh_all, xl_all = tiles["xh_all"], tiles["xl_all"]
            th_all, tl_all = tiles["th_all"], tiles["tl_all"]

            # MT groups stored pairwise: mt_pair[q] = [125, 2*XC] holding
            # groups 2q (cols 0:XC) and 2q+1 (cols XC:2XC), so one 3D AP
            # can cover both groups' windows for the dual-column abs op.
            mt_pair = []
            for q in range(NG // 2):
                mt_pair.append(
                    cpool.tile([PG, 2 * XC], F32, name=f"mt_p{q}", tag=f"mtp{q}"))
            for g in range(NG):
                mp = mpspool.tile([PG, XC], F32, name="mp", tag="mp")
                for it in range(4):
                    tsl = slice(it * O * K + g * PG, it * O * K + (g + 1) * PG)
                    xsl = slice(it * XC, (it + 1) * XC)
                    for term, (tw, xw) in enumerate(
                            ((th_all, xh_all), (th_all, xl_all),
                             (tl_all, xh_all))):
                        nc.tensor.matmul(
                            mp[:],
                            tw[:, tsl],
                            xw[:, xsl],
                            start=(it == 0 and term == 0),
                            stop=(it == 3 and term == 2),
                        )
                dst = mt_pair[g // 2][:, (g % 2) * XC:(g % 2) * XC + XC]
                if g % 2 == 0:
                    nc.vector.tensor_copy(dst, mp[:])
                else:
                    nc.scalar.copy(dst, mp[:])

            def mt_win(g, a, b):
                return mt_pair[g // 2][:, (g % 2) * XC + a:(g % 2) * XC + b]

            def mt_col(g, j):
                return mt_win(g, j, j + 1)

            ob_sb = cpool.tile([128, JB], F32, name="ob_sb")
            eaca_sb = cpool.tile([128, B], F32, name="eaca_sb")
            eacb_sb = cpool.tile([128, B], F32, name="eacb_sb")
            nc.vector.memset(eaca_sb[:], 0.0)
            nc.vector.memset(eacb_sb[:], 0.0)

            def emit_mm(np_t, g, mov):
                nc.tensor.matmul(
                    np_t[32 * g:32 * g + 32, :],
                    bo_sb[:],
                    mov,
                    start=True,
                    stop=True,
                    tile_position=(0, 32 * g),
                    skip_group_check=True,
                )

            def emit_abs_single(j, g, np_t):
                w0 = j + 1
                win = mt_win(g, w0, w0 + W)
                col = mt_col(g, j)
                dg = dgpool.tile([PG, W], F16, name="dg", tag="dg")
                if sched[4 * j + g] == "S":
                    nc.scalar.activation(
                        out=dg[:],
                        in_=win,
                        func=mybir.ActivationFunctionType.Abs,
                        bias=col,
                        scale=-1.0,
                    )
                else:
                    nc.vector._custom_dve(
                        ABS_DIFF_COL, out=dg[:], in0=win, s0=col,
                    )
                emit_mm(np_t, g, dg[:])

            def emit_pair(j, q, np_t):
                w0 = j + 1
                g0, g1 = 2 * q, 2 * q + 1
                if sched[4 * j + g0] != "S" and sched[4 * j + g1] != "S":
                    # dual-column op: both groups' windows in one call
                    win2 = mt_pair[q][:].rearrange(
                        "p (s c) -> p s c", s=2)[:, :, w0:w0 + W]
                    dg2 = dg2pool.tile([PG, 2 * W], F16, name="dg2",
                                       tag="dg2")
                    nc.vector._custom_dve(
                        ABS_DIFF_2COL,
                        out=dg2[:].rearrange("p (s c) -> p s c", s=2),
                        in0=win2,
                        s0=mt_col(g0, j),
                        s1=mt_col(g1, j),
                    )
                    emit_mm(np_t, g0, dg2[:, 0:W])
                    emit_mm(np_t, g1, dg2[:, W:2 * W])
                else:
                    emit_abs_single(j, g0, np_t)
                    emit_abs_single(j, g1, np_t)

            def emit_producers(j):
                np_t = npspool.tile([128, W], F32, name="np_t", tag="norm")
                emit_pair(j, 0, np_t)
                emit_pair(j, 1, np_t)
                return np_t

            def emit_consumer(j, np_t):
                w0 = j + 1
                ep = eppool.tile([128, W], F32, name="ep", tag="exp")
                nc.scalar.activation(
                    out=ep[:],
                    in_=np_t[:],
                    func=mybir.ActivationFunctionType.Exp,
                    scale=-1.0,
                    accum_out=ob_sb[:, j:j + 1],
                )
                dst = eaca_sb if j % 2 == 0 else eacb_sb
                eng = nc.vector if j >= JB - 4 else nc.gpsimd
                eng.tensor_tensor(
                    out=dst[:, w0:w0 + W],
                    in0=dst[:, w0:w0 + W],
                    in1=ep[:],
                    op=mybir.AluOpType.add,
                )

            def emit_extra_batch():
                # d=256 pairs: |MT[:, j+256] - MT[:, j]| for j = 0..63,
                # masked to cores 0-3 via EXB (exp(-norm + exb)).
                npx = npxpool.tile([128, JB], F32, name="npx", tag="normx")
                for g in range(NG):
                    tmp = dgpool.tile([PG, JB], F32, name="tmpx", tag="tmpx")
                    nc.gpsimd.tensor_tensor(
                        out=tmp[:],
                        in0=mt_win(g, 256, 256 + JB),
                        in1=mt_win(g, 0, JB),
                        op=mybir.AluOpType.subtract,
                    )
                    dgx = dgpool.tile([PG, JB], F16, name="dgx", tag="dgx")
                    nc.vector._custom_dve(
                        ABS_DIFF_COL, out=dgx[:], in0=tmp[:], s0=0.0,
                    )
                    nc.tensor.matmul(
                        npx[32 * g:32 * g + 32, :],
                        bo_sb[:],
                        dgx[:],
                        start=True,
                        stop=True,
                        tile_position=(0, 32 * g),
                        skip_group_check=True,
                    )
                expx = cpool.tile([128, JB], F32, name="expx")
                nc.scalar.activation(
                    out=expx[:],
                    in_=npx[:],
                    func=mybir.ActivationFunctionType.Exp,
                    scale=-1.0,
                    bias=exb_sb[:, 0:1],
                )
                return expx

            # Warmup: emit pair0 producers for the first WARM j's before any
            # pair1 work — the strict-FIFO Vector queue otherwise head-of-line
            # blocks on mt_pair1's copy while ready pair0 ops wait behind it.
            WARM = 4
            pending = []
            expx = None
            for j in range(WARM):
                np_t = npspool.tile([128, W], F32, name="np_t", tag="norm")
                emit_pair(j, 0, np_t)
                pending.append((j, np_t))
            for j, np_t in pending:
                emit_pair(j, 1, np_t)
            for j in range(WARM, JB):
                pending.append((j, emit_producers(j)))
                if j == 36:
                    expx = emit_extra_batch()
                if len(pending) > PIPE_DEPTH:
                    jc, npc = pending.pop(0)
                    emit_consumer(jc, npc)
            for jc, npc in pending:
                emit_consumer(jc, npc)

            nc.vector.tensor_tensor(
                out=ob_sb[:],
                in0=ob_sb[:],
                in1=expx[:],
                op=mybir.AluOpType.add,
            )
            nc.vector.tensor_tensor(
                out=eaca_sb[:, 256:256 + JB],
                in0=eaca_sb[:, 256:256 + JB],
                in1=expx[:],
                op=mybir.AluOpType.add,
            )

            nc.sync.dma_start(out=ob[:], in_=ob_sb[:])
            nc.scalar.dma_start(out=eaca[:], in_=eaca_sb[:, 0:XC])
            nc.sync.dma_start(out=eacb[:], in_=eacb_sb[:, 0:XC])

    nc.compile()
    return nc


_NC_CACHE = None


def _get_nc():
    global _NC_CACHE
    if _NC_CACHE is None:
        _NC_CACHE = _build_nc()
    return _NC_CACHE


def _make_consts():
    bo = np.zeros((PG, 32), dtype=np.float16)
    for p in range(PG):
        bo[p, p // K] = 1.0
    return bo


def _in_maps(x, T):
    import ml_dtypes
    bf16 = ml_dtypes.bfloat16
    bo = _make_consts()
    tt = np.ascontiguousarray(np.asarray(T, np.float32).reshape(IN_F, O * K))
    x = np.asarray(x, np.float32)
    th = tt.astype(bf16)
    tl = (tt - th.astype(np.float32)).astype(bf16)
    maps = []
    for c in range(NCORES):
        xtc = np.ascontiguousarray(np.roll(x, -JB * c, axis=0).T[:, :XC])
        xhc = xtc.astype(bf16)
        xlc = (xtc - xhc.astype(np.float32)).astype(bf16)
        exbc = np.full((128, 1), 0.0 if c < 4 else -40.0, dtype=np.float32)
        maps.append({"XH": xhc, "XL": xlc, "TH": th, "TL": tl,
                     "BO": bo, "BON": -bo, "EXB": exbc})
    return maps


def _assemble(results):
    out = np.zeros((B, O), dtype=np.float64)
    for c in range(NCORES):
        obc = results[c]["OB"]                      # [128, JB] row sums
        eacc = np.zeros((128, B), dtype=np.float64)
        eacc[:, :XC] = results[c]["EACA"].astype(np.float64) + \
            results[c]["EACB"].astype(np.float64)
        rows = (np.arange(B) + JB * c) % B
        for g in range(NG):
            out[JB * c:JB * (c + 1), OG * g:OG * (g + 1)] += \
                obc[32 * g:32 * g + OG, :].T.astype(np.float64)
            out[rows, OG * g:OG * (g + 1)] += \
                eacc[32 * g:32 * g + OG, :].T
    return out.astype(np.float32)


def kernel(x: np.ndarray, T: np.ndarray) -> np.ndarray:
    x = np.ascontiguousarray(np.asarray(x, dtype=np.float32))
    T = np.ascontiguousarray(np.asarray(T, dtype=np.float32))
    assert x.shape == (B, IN_F) and T.shape == (IN_F, O, K)

    nc = _get_nc()
    res = run_bass_kernel_spmd(nc, _in_maps(x, T), list(range(NCORES)))
    return _assemble(res.results)


if __name__ == "__main__":
    rng = np.random.default_rng(0)
    x = rng.standard_normal((B, IN_F), dtype=np.float32)
    T = rng.standard_normal((IN_F, O, K), dtype=np.float32)
    out = kernel(x, T)
    print("out", out.shape, out.dtype, np.abs(out).max())
